# revision 27
# baseline (speedup 1.0000x reference)
"""Bass/Trainium2 kernel for nn_BioInspiredESN (8-core SPMD).

Sharding (one TRN2 chip, 8 NeuronCores):
  - Stage 1: W_core / W_shell / W_feedback row-split 8 ways; every core holds
    the replicated reservoir state and computes its slice of core_pred1 /
    shell_pred / fb_pred.
  - AllGather #1: per-core shell_pred/fb_pred slices.
  - Stage 2: W_shell_core / W_feedback_core row-split by core-region rows;
    each core contracts against the gathered shell/fb predictions.
  - Stage 3: leak + harmonic-oscillator + dopamine update on the core's own
    slice of s (elementwise, f32).
  - AllGather #2: per-core s slices -> full s everywhere.
  - Stage 4: W_ih/W_hh row-split by LSTM hidden unit (each core owns the
    i/f/g/o gate rows of 128 hidden units); LSTM cell computed locally; the
    final W_out dot is a per-core partial the host sums.

GEMV orientation: batch=1, so every matmul makes the *state vector* the
stationary operand (a [128,1] LDWEIGHTS, essentially free) and streams the
*weight* tiles through the moving-operand port as N<=512 matmuls.  Outputs
land as single-partition PSUM rows; a handful of PE transposes (matmul
against an identity) restore the [128, C] column layout used by the
elementwise engines.  Weights stream as bf16 (validated ~1e-3 end-to-end
rel. error); all accumulation and state math is f32.

Layout conventions:
  cols:  a length-L vector as [128, ceil(L/128)], element (p,c) = v[c*128+p]
  rows:  a length-L vector as [1, L] on one partition (matmul-row output),
         or [C, 128] with chunk c on partition c (allgather shards)
"""

import numpy as np
import ml_dtypes

P = 128
NCORES = 8
R, CORE, SHELL, FB, H = 8192, 4915, 2457, 820, 1024
LEAK = 0.1

CORE_COLS, SHELL_COLS, FB_COLS = 5, 3, 1
OWN_COLS = CORE_COLS + SHELL_COLS + FB_COLS       # 9
M_CORE = CORE_COLS * P                            # 640
SHELL_OWN = 320
KT_CORE, KT_SHELL, KT_FB = 39, 20, 7
KT_X = 8 * 4                                      # 32 (3 shell + 1 fb per block)
KT_IH = 8 * OWN_COLS                              # 72
KT_HH = 8

BF16 = True
WNP = ml_dtypes.bfloat16 if BF16 else np.float32

CH_CORE = 8
CH_X = 8
CH_IH = 36

# packed-constant column/row offsets
P16_STC, P16_STS, P16_STF, P16_HCL, P16_IDN = 0, 39, 59, 66, 74
P16_N = 74 + P
P32_STO, P32_OMG, P32_GMM, P32_DOB, P32_IDN = 0, 9, 18, 27, 28
P32_N = 28 + P
PR_WIN, PR_CON, PR_BIO, PR_BHO, PR_WOU, PR_XB = 0, 640, 768, 1280, 1792, 1920
PR_N = 1921


def _split(n):
    base, rem = divmod(n, NCORES)
    return [base + 1] * rem + [base] * (NCORES - rem)


CC, SC, FC = _split(CORE), _split(SHELL), _split(FB)
C_OFF = np.concatenate([[0], np.cumsum(CC)]).astype(int)
S_OFF = np.concatenate([[0], np.cumsum(SC)]).astype(int)
F_OFF = np.concatenate([[0], np.cumsum(FC)]).astype(int)


def _cols(v, ncols, dtype=np.float32):
    v = np.asarray(v, np.float32).ravel()
    out = np.zeros(ncols * P, np.float32)
    out[: v.size] = v
    return np.ascontiguousarray(out.reshape(ncols, P).T).astype(dtype)


def _row(v, n, dtype=np.float32):
    v = np.asarray(v, np.float32).ravel()
    out = np.zeros((1, n), np.float32)
    out[0, : v.size] = v
    return out.astype(dtype)


def _chunks(total_kt, ch):
    out, k = [], 0
    while k < total_kt:
        out.append((k, min(ch, total_kt - k)))
        k += ch
    return out


def _prep_in_maps(inputs):
    f32 = np.float32
    x = np.asarray(inputs["x"], f32)
    state = np.asarray(inputs["state"], f32).ravel()
    h = np.asarray(inputs["lstm_hidden"], f32).ravel()
    c = np.asarray(inputs["lstm_cell"], f32).ravel()
    W_in = np.asarray(inputs["W_in"], f32)
    W_core = np.asarray(inputs["W_core"], f32)
    W_shell = np.asarray(inputs["W_shell"], f32)
    W_fb = np.asarray(inputs["W_feedback"], f32)
    W_sc = np.asarray(inputs["W_shell_core"], f32)
    W_fc = np.asarray(inputs["W_feedback_core"], f32)
    W_ih = np.asarray(inputs["W_ih"], f32)
    W_hh = np.asarray(inputs["W_hh"], f32)
    b_ih = np.asarray(inputs["b_ih"], f32)
    b_hh = np.asarray(inputs["b_hh"], f32)
    W_out = np.asarray(inputs["W_out"], f32)
    dop = np.float32(inputs["dopamine"])
    omega = np.asarray(inputs["omega"], f32)
    gamma = np.asarray(inputs["gamma"], f32)

    st_core, st_shell, st_fb = state[:CORE], state[CORE:CORE + SHELL], state[CORE + SHELL:]
    om_c, om_s, om_f = omega[:CORE], omega[CORE:CORE + SHELL], omega[CORE + SHELL:]
    gm_c, gm_s, gm_f = gamma[:CORE], gamma[CORE:CORE + SHELL], gamma[CORE + SHELL:]

    cpack16 = np.concatenate([
        _cols(st_core, KT_CORE), _cols(st_shell, KT_SHELL),
        _cols(st_fb, KT_FB), _cols(h, KT_HH),
        np.eye(P, dtype=f32)], axis=1).astype(WNP)

    def own9(v_c, v_s, v_f, i):
        return np.concatenate([
            _cols(v_c[C_OFF[i]:C_OFF[i] + CC[i]], CORE_COLS),
            _cols(v_s[S_OFF[i]:S_OFF[i] + SC[i]], SHELL_COLS),
            _cols(v_f[F_OFF[i]:F_OFF[i] + FC[i]], FB_COLS),
        ], axis=1)

    in_maps = []
    gate_rows = np.concatenate([np.arange(g * H, g * H + P) for g in range(4)])
    for i in range(NCORES):
        m = {}
        m["cpack16"] = cpack16
        m["cpack32"] = np.concatenate([
            own9(st_core, st_shell, st_fb, i),
            own9(om_c, om_s, om_f, i),
            own9(gm_c, gm_s, gm_f, i),
            np.full((P, 1), dop, f32),
            np.eye(P, dtype=f32)], axis=1)
        m["rpack32"] = np.concatenate([
            _row(W_in[C_OFF[i]:C_OFF[i] + CC[i], 0], M_CORE),
            _row(c[i * P:(i + 1) * P], P),
            _row(b_ih[gate_rows + i * P], 512),
            _row(b_hh[gate_rows + i * P], 512),
            _row(W_out[0, i * P:(i + 1) * P], P),
            np.full((1, 1), x[0], f32)], axis=1)

        # ---- stage-1 weights (K on rows, own outputs on cols)
        w = np.zeros((KT_CORE * P, M_CORE), np.float32)
        w[:CORE, :CC[i]] = W_core[C_OFF[i]:C_OFF[i] + CC[i], :].T
        m["wc"] = w.astype(WNP)

        w = np.zeros((KT_SHELL * P, SHELL_OWN), np.float32)
        w[:SHELL, :SC[i]] = W_shell[S_OFF[i]:S_OFF[i] + SC[i], :].T
        m["ws"] = w.astype(WNP)

        w = np.zeros((KT_FB * P, P), np.float32)
        w[:FB, :FC[i]] = W_fb[F_OFF[i]:F_OFF[i] + FC[i], :].T
        m["wf"] = w.astype(WNP)

        # ---- stage-2 combined cross weights in allgather-1 coordinates:
        # contraction tile kt = cslot*8 + j (cslot 0-2 shell chunks, 3 fb)
        w = np.zeros((KT_X * P, M_CORE), np.float32)
        wsc = W_sc[C_OFF[i]:C_OFF[i] + CC[i], :]
        wfc = W_fc[C_OFF[i]:C_OFF[i] + CC[i], :]
        for j in range(NCORES):
            for cs in range(3):
                lo = cs * P
                n = min(P, max(0, SC[j] - lo))
                if n:
                    w[(cs * 8 + j) * P:(cs * 8 + j) * P + n, :CC[i]] = \
                        wsc[:, S_OFF[j] + lo:S_OFF[j] + lo + n].T
            w[(3 * 8 + j) * P:(3 * 8 + j) * P + FC[j], :CC[i]] = \
                wfc[:, F_OFF[j]:F_OFF[j] + FC[j]].T
        m["wx"] = w.astype(WNP)

        # ---- stage-4 weights in allgather-2 coordinates
        wih_own = W_ih[gate_rows + i * P]             # (512, 8192), row m = g*128+u
        w = np.zeros((KT_IH * P, 512), np.float32)
        for j in range(NCORES):
            b = j * (OWN_COLS * P)
            w[b:b + CC[j], :] = wih_own[:, C_OFF[j]:C_OFF[j] + CC[j]].T
            w[b + 640:b + 640 + SC[j], :] = wih_own[:, CORE + S_OFF[j]:CORE + S_OFF[j] + SC[j]].T
            w[b + 1024:b + 1024 + FC[j], :] = wih_own[:, CORE + SHELL + F_OFF[j]:CORE + SHELL + F_OFF[j] + FC[j]].T
        m["wih"] = w.astype(WNP)

        m["whh"] = np.ascontiguousarray(W_hh[gate_rows + i * P].T).astype(WNP)

        in_maps.append(m)
    return in_maps


def _build_nc():
    from concourse import bacc, mybir
    import concourse.tile as tile

    f32 = mybir.dt.float32
    dtw = mybir.dt.bfloat16 if BF16 else mybir.dt.float32
    AF = mybir.ActivationFunctionType
    ALU = mybir.AluOpType
    AX = mybir.AxisListType

    nc = bacc.Bacc("TRN2", target_bir_lowering=False, debug=False,
                   num_devices=NCORES)

    def din(name, shape, dt=f32):
        return nc.dram_tensor(name, list(shape), dt, kind="ExternalInput").ap()

    cpack16 = din("cpack16", (P, P16_N), dtw)
    cpack32 = din("cpack32", (P, P32_N))
    rpack32 = din("rpack32", (1, PR_N))
    wc = din("wc", (KT_CORE * P, M_CORE), dtw)
    ws = din("ws", (KT_SHELL * P, SHELL_OWN), dtw)
    wf = din("wf", (KT_FB * P, P), dtw)
    wx = din("wx", (KT_X * P, M_CORE), dtw)
    wih = din("wih", (KT_IH * P, 512), dtw)
    whh = din("whh", (KT_HH * P, 512), dtw)

    s_out = nc.dram_tensor("s_out", [OWN_COLS, P], f32, kind="ExternalOutput").ap()
    h_out = nc.dram_tensor("h_out", [1, P], f32, kind="ExternalOutput").ap()
    c_out = nc.dram_tensor("c_out", [1, P], f32, kind="ExternalOutput").ap()
    po_out = nc.dram_tensor("po_out", [1, 1], f32, kind="ExternalOutput").ap()

    rg = [list(range(NCORES))]

    with tile.TileContext(nc) as tc:
        with (
            tc.tile_pool(name="const", bufs=1) as cp,
            tc.tile_pool(name="wcp", bufs=2) as wcp,
            tc.tile_pool(name="wxp", bufs=2) as wxp,
            tc.tile_pool(name="wihp", bufs=3) as wihp,
            tc.tile_pool(name="work", bufs=1) as wk,
            tc.tile_pool(name="psum", bufs=1, space="PSUM") as pp,
            tc.tile_pool(name="dram", bufs=1, space="DRAM") as dp,
        ):
            # ---- packed constants (3 DMAs); shell weights right behind them
            c16 = cp.tile([P, P16_N], dtw, tag="c16")
            nc.sync.dma_start(c16[:], cpack16)
            c_ws = cp.tile([P, KT_SHELL * SHELL_OWN], dtw, tag="c_ws")
            nc.sync.dma_start(
                c_ws[:].rearrange("p (t m) -> p t m", m=SHELL_OWN),
                ws.rearrange("(t p) m -> p t m", p=P))
            c_wf = cp.tile([P, KT_FB * P], dtw, tag="c_wf")
            nc.sync.dma_start(
                c_wf[:].rearrange("p (t m) -> p t m", m=P),
                wf.rearrange("(t p) m -> p t m", p=P))
            c32 = cp.tile([P, P32_N], f32, tag="c32")
            nc.sync.dma_start(c32[:], cpack32)
            cr = cp.tile([1, PR_N], f32, tag="cr")
            nc.sync.dma_start(cr[:], rpack32)

            c_stc = c16[:, P16_STC:P16_STC + KT_CORE]
            c_sts = c16[:, P16_STS:P16_STS + KT_SHELL]
            c_stf = c16[:, P16_STF:P16_STF + KT_FB]
            c_hcl = c16[:, P16_HCL:P16_HCL + KT_HH]
            c_idn16 = c16[:, P16_IDN:P16_IDN + P]
            c_sto = c32[:, P32_STO:P32_STO + OWN_COLS]
            c_omg = c32[:, P32_OMG:P32_OMG + OWN_COLS]
            c_gmm = c32[:, P32_GMM:P32_GMM + OWN_COLS]
            c_dob = c32[:, P32_DOB:P32_DOB + 1]
            c_idn32 = c32[:, P32_IDN:P32_IDN + P]
            c_win = cr[:, PR_WIN:PR_WIN + M_CORE]
            c_con = cr[:, PR_CON:PR_CON + P]
            c_bio = cr[:, PR_BIO:PR_BIO + 512]
            c_bho = cr[:, PR_BHO:PR_BHO + 512]
            c_wou = cr[:, PR_WOU:PR_WOU + P]
            c_xb = cr[:, PR_XB:PR_XB + 1]

            # ---- PSUM row tiles (one bank each: exactly one start/stop per bank)
            ps_g = pp.tile([1, 512], f32, tag="ps_g")
            ps_s = pp.tile([1, SHELL_OWN], f32, tag="row", bufs=5)
            ps_f = pp.tile([1, P], f32, tag="row", bufs=5)
            ps_ca = pp.tile([1, 512], f32, tag="row", bufs=5)
            ps_cb = pp.tile([1, P], f32, tag="row", bufs=5)

            # ---- stage 1: shell/fb first so every core reaches allgather-1 early
            for kt in range(KT_SHELL):
                nc.tensor.matmul(ps_s[0:1, :], c_sts[:, kt:kt + 1],
                                 c_ws[:, kt * SHELL_OWN:(kt + 1) * SHELL_OWN],
                                 start=(kt == 0), stop=(kt == KT_SHELL - 1))
            for kt in range(KT_FB):
                nc.tensor.matmul(ps_f[0:1, :], c_stf[:, kt:kt + 1],
                                 c_wf[:, kt * P:(kt + 1) * P],
                                 start=(kt == 0), stop=(kt == KT_FB - 1))

            # allgather-1 shard row: slots 0-2 shell_pred chunks, slot 3 fb_pred
            # (bf16 for the wire + an f32 twin for the local pred transpose)
            shard = wk.tile([1, 512], dtw, tag="shard")
            nc.vector.memset(shard[:], 0.0)
            srow32 = wk.tile([1, 512], f32, tag="srow32")
            nc.vector.memset(srow32[:], 0.0)
            for dst in (shard, srow32):
                nc.scalar.activation(dst[0:1, 0:P], ps_s[0:1, 0:P], AF.Tanh)
                nc.scalar.activation(dst[0:1, P:2 * P], ps_s[0:1, P:2 * P], AF.Tanh)
                nc.scalar.activation(dst[0:1, 2 * P:SHELL_OWN],
                                     ps_s[0:1, 2 * P:SHELL_OWN], AF.Tanh)
                nc.scalar.activation(dst[0:1, 3 * P:4 * P], ps_f[0:1, :], AF.Tanh)

            ag1_src = dp.tile([1, 512], dtw, tag="ag1_src")
            ag1_dst = dp.tile([NCORES, 512], dtw, tag="ag1_dst")
            nc.scalar.dma_start(ag1_src[:], shard[:])
            nc.gpsimd.collective_compute(
                "AllGather", ALU.bypass, replica_groups=rg,
                ins=[ag1_src.opt()], outs=[ag1_dst.opt()])
            ag1_g = wk.tile([NCORES, 512], dtw, tag="ag1_g")
            nc.scalar.dma_start(ag1_g[:], ag1_dst[:])

            # ---- W_hh @ h partial gates (opens the ps_g accumulation)
            c_whh = cp.tile([P, KT_HH * 512], dtw, tag="c_whh")
            nc.sync.dma_start(
                c_whh[:].rearrange("p (t m) -> p t m", m=512),
                whh.rearrange("(t p) m -> p t m", p=P))
            for kt in range(KT_HH):
                nc.tensor.matmul(ps_g[0:1, :], c_hcl[:, kt:kt + 1],
                                 c_whh[:, kt * 512:(kt + 1) * 512],
                                 start=(kt == 0), stop=False)

            # ---- cheap precomputes
            a_t = wk.tile([P, OWN_COLS], f32, tag="a_t")      # 1 + 0.1*omega
            nc.vector.tensor_scalar(a_t[:], c_omg, 0.1, 1.0, ALU.mult, ALU.add)
            b_t = wk.tile([P, OWN_COLS], f32, tag="b_t")      # 0.1*gamma
            nc.vector.tensor_scalar_mul(b_t[:], c_gmm, 0.1)
            st9 = wk.tile([P, OWN_COLS], f32, tag="st9")      # 0.9*state_own
            nc.vector.tensor_scalar_mul(st9[:], c_sto, 1.0 - LEAK)
            bsum = wk.tile([1, 512], f32, tag="bsum")
            nc.vector.tensor_add(bsum[:], c_bio, c_bho)
            rin = wk.tile([1, M_CORE], f32, tag="rin")        # W_in[:CORE]*x
            nc.vector.tensor_scalar(rin[:], c_win, c_xb[0:1, 0:1], None, ALU.mult)

            # ---- stage-1 core GEMV (streams while allgather-1 is in flight)
            for (kt0, nkt) in _chunks(KT_CORE, CH_CORE):
                t = wcp.tile([P, CH_CORE * M_CORE], dtw, tag="wc")
                nc.sync.dma_start(
                    t[:, : nkt * M_CORE].rearrange("p (t m) -> p t m", m=M_CORE),
                    wc[kt0 * P:(kt0 + nkt) * P, :].rearrange("(t p) m -> p t m", p=P))
                for k in range(nkt):
                    kt = kt0 + k
                    xcol = c_stc[:, kt:kt + 1]
                    nc.tensor.matmul(ps_ca[0:1, :], xcol,
                                     t[:, k * M_CORE: k * M_CORE + 512],
                                     start=(kt == 0), stop=(kt == KT_CORE - 1))
                    nc.tensor.matmul(ps_cb[0:1, :], xcol,
                                     t[:, k * M_CORE + 512:(k + 1) * M_CORE],
                                     start=(kt == 0), stop=(kt == KT_CORE - 1))

            # core_pred1 = tanh(psum + rin), kept in row layout
            crow = wk.tile([1, M_CORE], f32, tag="crow")
            nc.vector.tensor_add(crow[0:1, 0:512], ps_ca[0:1, :], rin[0:1, 0:512])
            nc.vector.tensor_add(crow[0:1, 512:M_CORE], ps_cb[0:1, :],
                                 rin[0:1, 512:M_CORE])
            nc.scalar.activation(crow[:], crow[:], AF.Tanh)

            # ---- transpose gathered shards -> [128, 32] columns (col = cs*8+j)
            ps_t3 = pp.tile([P, 4 * NCORES], dtw, tag="tr", bufs=2)
            for cs in range(4):
                nc.tensor.matmul(ps_t3[:, cs * 8:(cs + 1) * 8],
                                 ag1_g[:, cs * P:(cs + 1) * P],
                                 c_idn16[0:NCORES, 0:NCORES],
                                 is_transpose=True, start=True, stop=True)
            ag1_sb = wk.tile([P, 4 * NCORES], dtw, tag="ag1_sb")
            nc.vector.tensor_copy(ag1_sb[:], ps_t3[:])

            # ---- stage 2: cross contributions (contract full shell/fb preds)
            ps_sca = pp.tile([1, 512], f32, tag="row", bufs=5)
            ps_scb = pp.tile([1, P], f32, tag="row", bufs=5)
            ps_fca = pp.tile([1, 512], f32, tag="row", bufs=5)
            ps_fcb = pp.tile([1, P], f32, tag="row", bufs=5)
            for (kt0, nkt) in _chunks(KT_X, CH_X):
                t = wxp.tile([P, CH_X * M_CORE], dtw, tag="wx")
                nc.sync.dma_start(
                    t[:, : nkt * M_CORE].rearrange("p (t m) -> p t m", m=M_CORE),
                    wx[kt0 * P:(kt0 + nkt) * P, :].rearrange("(t p) m -> p t m", p=P))
                for k in range(nkt):
                    kt = kt0 + k
                    xcol = ag1_sb[:, kt:kt + 1]      # col = cslot*8 + j
                    if kt < 24:
                        pa, pb, st, sp = ps_sca, ps_scb, kt == 0, kt == 23
                    else:
                        pa, pb, st, sp = ps_fca, ps_fcb, kt == 24, kt == KT_X - 1
                    nc.tensor.matmul(pa[0:1, :], xcol,
                                     t[:, k * M_CORE: k * M_CORE + 512],
                                     start=st, stop=sp)
                    nc.tensor.matmul(pb[0:1, :], xcol,
                                     t[:, k * M_CORE + 512:(k + 1) * M_CORE],
                                     start=st, stop=sp)

            # core_pred += tanh(sc) + tanh(fc)   (rows)
            tsc = wk.tile([1, M_CORE], f32, tag="tsc")
            nc.scalar.activation(tsc[0:1, 0:512], ps_sca[0:1, :], AF.Tanh)
            nc.scalar.activation(tsc[0:1, 512:M_CORE], ps_scb[0:1, :], AF.Tanh)
            tfc = wk.tile([1, M_CORE], f32, tag="tfc")
            nc.scalar.activation(tfc[0:1, 0:512], ps_fca[0:1, :], AF.Tanh)
            nc.scalar.activation(tfc[0:1, 512:M_CORE], ps_fcb[0:1, :], AF.Tanh)
            nc.vector.tensor_add(crow[:], crow[:], tsc[:])
            nc.vector.tensor_add(crow[:], crow[:], tfc[:])

            # ---- transpose pred into column layout
            ps_pred = pp.tile([P, CORE_COLS], f32, tag="tr", bufs=2)
            for cslot in range(CORE_COLS):
                nc.tensor.matmul(ps_pred[:, cslot:cslot + 1],
                                 crow[0:1, cslot * P:(cslot + 1) * P],
                                 c_idn32[0:1, 0:1],
                                 is_transpose=True, start=True, stop=True)
            ps_pred2 = pp.tile([P, 4], f32, tag="tr", bufs=2)
            for cs in range(4):
                nc.tensor.matmul(ps_pred2[:, cs:cs + 1],
                                 srow32[0:1, cs * P:(cs + 1) * P],
                                 c_idn32[0:1, 0:1],
                                 is_transpose=True, start=True, stop=True)

            # ---- stage 3: leak + harmonic oscillator + dopamine (columns)
            s_own = wk.tile([P, OWN_COLS], f32, tag="s_own")
            nc.vector.scalar_tensor_tensor(s_own[:, 0:CORE_COLS],
                                           ps_pred[:], LEAK,
                                           st9[:, 0:CORE_COLS],
                                           ALU.mult, ALU.add)
            nc.vector.scalar_tensor_tensor(s_own[:, CORE_COLS:OWN_COLS],
                                           ps_pred2[:], LEAK,
                                           st9[:, CORE_COLS:OWN_COLS],
                                           ALU.mult, ALU.add)
            t2 = wk.tile([P, OWN_COLS], f32, tag="t2")
            nc.scalar.activation(t2[:], s_own[:], AF.Tanh)
            t3 = wk.tile([P, OWN_COLS], f32, tag="t3")
            nc.vector.tensor_mul(t3[:], s_own[:], a_t[:])
            nc.vector.tensor_mul(t2[:], t2[:], b_t[:])
            nc.vector.tensor_sub(s_own[:], t3[:], t2[:])
            nc.vector.tensor_scalar(s_own[:], s_own[:], c_dob[:, 0:1], None, ALU.mult)

            # ---- s to rows; emit output + allgather-2 shard
            ps_ts = pp.tile([OWN_COLS, P], f32, tag="tr", bufs=2)
            nc.tensor.matmul(ps_ts[:], s_own[:], c_idn32[:, 0:P],
                             is_transpose=True, start=True, stop=True)
            s_rows = wk.tile([OWN_COLS, P], f32, tag="s_rows")
            nc.vector.tensor_copy(s_rows[:], ps_ts[:])
            nc.scalar.dma_start(s_out, s_rows[:])
            s_rows16 = wk.tile([OWN_COLS, P], dtw, tag="s_rows16")
            nc.vector.tensor_copy(s_rows16[:], ps_ts[:])

            ag2_src = dp.tile([OWN_COLS, P], dtw, tag="ag2_src")
            ag2_dst = dp.tile([OWN_COLS * NCORES, P], dtw, tag="ag2_dst")
            nc.scalar.dma_start(ag2_src[:], s_rows16[:])
            nc.gpsimd.collective_compute(
                "AllGather", ALU.bypass, replica_groups=rg,
                ins=[ag2_src.opt()], outs=[ag2_dst.opt()])
            ag2_g = wk.tile([OWN_COLS * NCORES, P], dtw, tag="ag2_g")
            nc.scalar.dma_start(ag2_g[:], ag2_dst[:])

            ps_t4 = pp.tile([P, OWN_COLS * NCORES], dtw, tag="tr", bufs=2)
            nc.tensor.matmul(ps_t4[:], ag2_g[:],
                             c_idn16[0:OWN_COLS * NCORES, 0:OWN_COLS * NCORES],
                             is_transpose=True, start=True, stop=True)
            ag2_sb = wk.tile([P, OWN_COLS * NCORES], dtw, tag="ag2_sb")
            nc.vector.tensor_copy(ag2_sb[:], ps_t4[:])

            # ---- stage 4: gates += W_ih @ s
            for (kt0, nkt) in _chunks(KT_IH, CH_IH):
                t = wihp.tile([P, CH_IH * 512], dtw, tag="wih")
                nc.sync.dma_start(
                    t[:, : nkt * 512].rearrange("p (t m) -> p t m", m=512),
                    wih[kt0 * P:(kt0 + nkt) * P, :].rearrange("(t p) m -> p t m", p=P))
                for k in range(nkt):
                    kt = kt0 + k
                    nc.tensor.matmul(ps_g[0:1, :], ag2_sb[:, kt:kt + 1],
                                     t[:, k * 512:(k + 1) * 512],
                                     start=False, stop=(kt == KT_IH - 1))

            # ---- LSTM cell for this core's 128 hidden units (row layout)
            g_row = wk.tile([1, 512], f32, tag="g_row")
            nc.vector.tensor_add(g_row[:], ps_g[0:1, :], bsum[:])
            acts = wk.tile([1, 512], f32, tag="acts")
            nc.scalar.activation(acts[0:1, 0:P], g_row[0:1, 0:P], AF.Sigmoid)
            nc.scalar.activation(acts[0:1, P:2 * P], g_row[0:1, P:2 * P], AF.Sigmoid)
            nc.scalar.activation(acts[0:1, 2 * P:3 * P], g_row[0:1, 2 * P:3 * P], AF.Tanh)
            nc.scalar.activation(acts[0:1, 3 * P:4 * P], g_row[0:1, 3 * P:4 * P], AF.Sigmoid)
            t5 = wk.tile([1, P], f32, tag="t5")
            nc.vector.tensor_mul(t5[:], acts[0:1, P:2 * P], c_con)       # f*c
            t6 = wk.tile([1, P], f32, tag="t6")
            nc.vector.tensor_mul(t6[:], acts[0:1, 0:P], acts[0:1, 2 * P:3 * P])
            c_t = wk.tile([1, P], f32, tag="c_t")
            nc.vector.tensor_add(c_t[:], t5[:], t6[:])                      # c_new
            tcn = wk.tile([1, P], f32, tag="tcn")
            nc.scalar.activation(tcn[:], c_t[:], AF.Tanh)
            h_t = wk.tile([1, P], f32, tag="h_t")
            nc.vector.tensor_mul(h_t[:], acts[0:1, 3 * P:4 * P], tcn[:])    # h_new
            hw = wk.tile([1, P], f32, tag="hw")
            nc.vector.tensor_mul(hw[:], h_t[:], c_wou)
            po_sb = wk.tile([1, 1], f32, tag="po_sb")
            nc.vector.reduce_sum(po_sb[:], hw[:], axis=AX.X)
            nc.scalar.dma_start(h_out, h_t[:])
            nc.scalar.dma_start(c_out, c_t[:])
            nc.scalar.dma_start(po_out, po_sb[:])

    nc.compile()
    return nc


_NC_CACHE = None


def _get_nc():
    global _NC_CACHE
    if _NC_CACHE is None:
        _NC_CACHE = _build_nc()
    return _NC_CACHE


def _unshard(results, b_out):
    s_full = np.zeros(R, np.float32)
    for i in range(NCORES):
        flat = results[i]["s_out"].reshape(-1)
        s_full[C_OFF[i]:C_OFF[i] + CC[i]] = flat[:CC[i]]
        s_full[CORE + S_OFF[i]:CORE + S_OFF[i] + SC[i]] = flat[640:640 + SC[i]]
        s_full[CORE + SHELL + F_OFF[i]:CORE + SHELL + F_OFF[i] + FC[i]] = \
            flat[1024:1024 + FC[i]]
    h_new = np.concatenate([results[i]["h_out"][0] for i in range(NCORES)])
    c_new = np.concatenate([results[i]["c_out"][0] for i in range(NCORES)])
    out = np.float32(sum(float(results[i]["po_out"][0, 0]) for i in range(NCORES)))
    output = (out + b_out.astype(np.float32)).reshape(1, 1)
    return (output.astype(np.float32),
            s_full.reshape(1, R),
            h_new.reshape(1, 1, H).astype(np.float32),
            c_new.reshape(1, 1, H).astype(np.float32))


def run(inputs, trace=False):
    from concourse import bass_utils
    nc = _get_nc()
    in_maps = _prep_in_maps(inputs)
    kw = {}
    if trace:
        kw = dict(trace=True, trace_cores=list(range(NCORES)))
    res = bass_utils.run_bass_kernel_spmd(
        nc, in_maps, core_ids=list(range(NCORES)), **kw)
    outs = _unshard(res.results, np.asarray(inputs["b_out"], np.float32))
    return outs, res


def kernel(**inputs):
    outs, _ = run(inputs, trace=False)
    return outs


# revision 33
# speedup vs baseline: 1.1441x; 1.1441x over previous
"""Bass/Trainium2 kernel for nn_BioInspiredESN (8-core SPMD).

Sharding (one TRN2 chip, 8 NeuronCores):
  - Stage 1: W_core / W_shell / W_feedback row-split 8 ways; every core holds
    the replicated reservoir state and computes its slice of core_pred1 /
    shell_pred / fb_pred.
  - AllGather #1: per-core shell_pred/fb_pred slices.
  - Stage 2: W_shell_core / W_feedback_core row-split by core-region rows;
    each core contracts against the gathered shell/fb predictions.
  - Stage 3: leak + harmonic-oscillator + dopamine update on the core's own
    slice of s (elementwise, f32).
  - AllGather #2: per-core s slices -> full s everywhere.
  - Stage 4: W_ih/W_hh row-split by LSTM hidden unit (each core owns the
    i/f/g/o gate rows of 128 hidden units); LSTM cell computed locally; the
    final W_out dot is a per-core partial the host sums.

GEMV orientation: batch=1, so every matmul makes the *state vector* the
stationary operand (a [128,1] LDWEIGHTS, essentially free) and streams the
*weight* tiles through the moving-operand port as N<=512 matmuls.  Outputs
land as single-partition PSUM rows; a handful of PE transposes (matmul
against an identity) restore the [128, C] column layout used by the
elementwise engines.  Weights stream as bf16 (validated ~1e-3 end-to-end
rel. error); all accumulation and state math is f32.

Layout conventions:
  cols:  a length-L vector as [128, ceil(L/128)], element (p,c) = v[c*128+p]
  rows:  a length-L vector as [1, L] on one partition (matmul-row output),
         or [C, 128] with chunk c on partition c (allgather shards)
"""

import numpy as np
import ml_dtypes

P = 128
NCORES = 8
R, CORE, SHELL, FB, H = 8192, 4915, 2457, 820, 1024
LEAK = 0.1

CORE_COLS, SHELL_COLS, FB_COLS = 5, 3, 1
OWN_COLS = CORE_COLS + SHELL_COLS + FB_COLS       # 9
M_CORE = CORE_COLS * P                            # 640
SHELL_OWN = 320
KT_CORE, KT_SHELL, KT_FB = 39, 20, 7
KT_X = 8 * 4                                      # 32 (3 shell + 1 fb per block)
KT_IH = 8 * OWN_COLS                              # 72
KT_HH = 8

BF16 = True
WNP = ml_dtypes.bfloat16 if BF16 else np.float32

CH_CORE = 8
CH_X = 16
CH_IH = 36

# packed-constant column/row offsets
P16_STC, P16_STS, P16_STF, P16_HCL, P16_IDN = 0, 39, 59, 66, 74
P16_N = 74 + P
P32_STO, P32_OMG, P32_GMM, P32_DOB, P32_IDN = 0, 9, 18, 27, 28
P32_N = 28 + P
PR_WIN, PR_CON, PR_BIO, PR_BHO, PR_WOU, PR_XB = 0, 640, 768, 1280, 1792, 1920
PR_N = 1921


def _split(n):
    base, rem = divmod(n, NCORES)
    return [base + 1] * rem + [base] * (NCORES - rem)


CC, SC, FC = _split(CORE), _split(SHELL), _split(FB)
C_OFF = np.concatenate([[0], np.cumsum(CC)]).astype(int)
S_OFF = np.concatenate([[0], np.cumsum(SC)]).astype(int)
F_OFF = np.concatenate([[0], np.cumsum(FC)]).astype(int)


def _cols(v, ncols, dtype=np.float32):
    v = np.asarray(v, np.float32).ravel()
    out = np.zeros(ncols * P, np.float32)
    out[: v.size] = v
    return np.ascontiguousarray(out.reshape(ncols, P).T).astype(dtype)


def _row(v, n, dtype=np.float32):
    v = np.asarray(v, np.float32).ravel()
    out = np.zeros((1, n), np.float32)
    out[0, : v.size] = v
    return out.astype(dtype)


def _chunks(total_kt, ch):
    out, k = [], 0
    while k < total_kt:
        out.append((k, min(ch, total_kt - k)))
        k += ch
    return out


def _prep_in_maps(inputs):
    f32 = np.float32
    x = np.asarray(inputs["x"], f32)
    state = np.asarray(inputs["state"], f32).ravel()
    h = np.asarray(inputs["lstm_hidden"], f32).ravel()
    c = np.asarray(inputs["lstm_cell"], f32).ravel()
    W_in = np.asarray(inputs["W_in"], f32)
    W_core = np.asarray(inputs["W_core"], f32)
    W_shell = np.asarray(inputs["W_shell"], f32)
    W_fb = np.asarray(inputs["W_feedback"], f32)
    W_sc = np.asarray(inputs["W_shell_core"], f32)
    W_fc = np.asarray(inputs["W_feedback_core"], f32)
    W_ih = np.asarray(inputs["W_ih"], f32)
    W_hh = np.asarray(inputs["W_hh"], f32)
    b_ih = np.asarray(inputs["b_ih"], f32)
    b_hh = np.asarray(inputs["b_hh"], f32)
    W_out = np.asarray(inputs["W_out"], f32)
    dop = np.float32(inputs["dopamine"])
    omega = np.asarray(inputs["omega"], f32)
    gamma = np.asarray(inputs["gamma"], f32)

    st_core, st_shell, st_fb = state[:CORE], state[CORE:CORE + SHELL], state[CORE + SHELL:]
    om_c, om_s, om_f = omega[:CORE], omega[CORE:CORE + SHELL], omega[CORE + SHELL:]
    gm_c, gm_s, gm_f = gamma[:CORE], gamma[CORE:CORE + SHELL], gamma[CORE + SHELL:]

    cpack16 = np.concatenate([
        _cols(st_core, KT_CORE), _cols(st_shell, KT_SHELL),
        _cols(st_fb, KT_FB), _cols(h, KT_HH),
        np.eye(P, dtype=f32)], axis=1).astype(WNP)

    def own9(v_c, v_s, v_f, i):
        return np.concatenate([
            _cols(v_c[C_OFF[i]:C_OFF[i] + CC[i]], CORE_COLS),
            _cols(v_s[S_OFF[i]:S_OFF[i] + SC[i]], SHELL_COLS),
            _cols(v_f[F_OFF[i]:F_OFF[i] + FC[i]], FB_COLS),
        ], axis=1)

    in_maps = []
    gate_rows = np.concatenate([np.arange(g * H, g * H + P) for g in range(4)])
    for i in range(NCORES):
        m = {}
        m["cpack16"] = cpack16
        m["cpack32"] = np.concatenate([
            own9(st_core, st_shell, st_fb, i),
            own9(om_c, om_s, om_f, i),
            own9(gm_c, gm_s, gm_f, i),
            np.full((P, 1), dop, f32),
            np.eye(P, dtype=f32)], axis=1)
        m["rpack32"] = np.concatenate([
            _row(W_in[C_OFF[i]:C_OFF[i] + CC[i], 0], M_CORE),
            _row(c[i * P:(i + 1) * P], P),
            _row(b_ih[gate_rows + i * P], 512),
            _row(b_hh[gate_rows + i * P], 512),
            _row(W_out[0, i * P:(i + 1) * P], P),
            np.full((1, 1), x[0], f32)], axis=1)

        # ---- stage-1 weights (K on rows, own outputs on cols)
        w = np.zeros((KT_CORE * P, M_CORE), np.float32)
        w[:CORE, :CC[i]] = W_core[C_OFF[i]:C_OFF[i] + CC[i], :].T
        m["wc"] = w.astype(WNP)

        w = np.zeros((KT_SHELL * P, SHELL_OWN), np.float32)
        w[:SHELL, :SC[i]] = W_shell[S_OFF[i]:S_OFF[i] + SC[i], :].T
        m["ws"] = w.astype(WNP)

        w = np.zeros((KT_FB * P, P), np.float32)
        w[:FB, :FC[i]] = W_fb[F_OFF[i]:F_OFF[i] + FC[i], :].T
        m["wf"] = w.astype(WNP)

        # ---- stage-2 combined cross weights in allgather-1 coordinates:
        # contraction tile kt = cslot*8 + j (cslot 0-2 shell chunks, 3 fb)
        w = np.zeros((KT_X * P, M_CORE), np.float32)
        wsc = W_sc[C_OFF[i]:C_OFF[i] + CC[i], :]
        wfc = W_fc[C_OFF[i]:C_OFF[i] + CC[i], :]
        for j in range(NCORES):
            for cs in range(3):
                lo = cs * P
                n = min(P, max(0, SC[j] - lo))
                if n:
                    w[(cs * 8 + j) * P:(cs * 8 + j) * P + n, :CC[i]] = \
                        wsc[:, S_OFF[j] + lo:S_OFF[j] + lo + n].T
            w[(3 * 8 + j) * P:(3 * 8 + j) * P + FC[j], :CC[i]] = \
                wfc[:, F_OFF[j]:F_OFF[j] + FC[j]].T
        m["wx"] = w.astype(WNP)

        # ---- stage-4 weights in allgather-2 coordinates
        wih_own = W_ih[gate_rows + i * P]             # (512, 8192), row m = g*128+u
        w = np.zeros((KT_IH * P, 512), np.float32)
        for j in range(NCORES):
            b = j * (OWN_COLS * P)
            w[b:b + CC[j], :] = wih_own[:, C_OFF[j]:C_OFF[j] + CC[j]].T
            w[b + 640:b + 640 + SC[j], :] = wih_own[:, CORE + S_OFF[j]:CORE + S_OFF[j] + SC[j]].T
            w[b + 1024:b + 1024 + FC[j], :] = wih_own[:, CORE + SHELL + F_OFF[j]:CORE + SHELL + F_OFF[j] + FC[j]].T
        m["wih"] = w.astype(WNP)

        m["whh"] = np.ascontiguousarray(W_hh[gate_rows + i * P].T).astype(WNP)

        in_maps.append(m)
    return in_maps


def _build_nc():
    from concourse import bacc, mybir
    import concourse.tile as tile

    f32 = mybir.dt.float32
    dtw = mybir.dt.bfloat16 if BF16 else mybir.dt.float32
    AF = mybir.ActivationFunctionType
    ALU = mybir.AluOpType
    AX = mybir.AxisListType

    nc = bacc.Bacc("TRN2", target_bir_lowering=False, debug=False,
                   num_devices=NCORES)

    def din(name, shape, dt=f32):
        return nc.dram_tensor(name, list(shape), dt, kind="ExternalInput").ap()

    cpack16 = din("cpack16", (P, P16_N), dtw)
    cpack32 = din("cpack32", (P, P32_N))
    rpack32 = din("rpack32", (1, PR_N))
    wc = din("wc", (KT_CORE * P, M_CORE), dtw)
    ws = din("ws", (KT_SHELL * P, SHELL_OWN), dtw)
    wf = din("wf", (KT_FB * P, P), dtw)
    wx = din("wx", (KT_X * P, M_CORE), dtw)
    wih = din("wih", (KT_IH * P, 512), dtw)
    whh = din("whh", (KT_HH * P, 512), dtw)

    s_out = nc.dram_tensor("s_out", [OWN_COLS, P], f32, kind="ExternalOutput").ap()
    h_out = nc.dram_tensor("h_out", [1, P], f32, kind="ExternalOutput").ap()
    c_out = nc.dram_tensor("c_out", [1, P], f32, kind="ExternalOutput").ap()
    po_out = nc.dram_tensor("po_out", [1, 1], f32, kind="ExternalOutput").ap()

    rg = [list(range(NCORES))]

    with tile.TileContext(nc) as tc:
        with (
            tc.tile_pool(name="const", bufs=1) as cp,
            tc.tile_pool(name="wcp", bufs=2) as wcp,
            tc.tile_pool(name="wxp", bufs=2) as wxp,
            tc.tile_pool(name="wihp", bufs=1) as wihp,
            tc.tile_pool(name="work", bufs=1) as wk,
            tc.tile_pool(name="psum", bufs=1, space="PSUM") as pp,
            tc.tile_pool(name="dram", bufs=1, space="DRAM") as dp,
        ):
            # ---- packed constants (3 DMAs); shell weights right behind them
            c16 = cp.tile([P, P16_N], dtw, tag="c16")
            nc.sync.dma_start(c16[:], cpack16)
            # ws and whh borrow wx-pool slots (they die before wx streams in)
            c_ws = wxp.tile([P, KT_SHELL * SHELL_OWN], dtw, tag="wx")
            nc.sync.dma_start(
                c_ws[:].rearrange("p (t m) -> p t m", m=SHELL_OWN),
                ws.rearrange("(t p) m -> p t m", p=P))
            c_wf = cp.tile([P, KT_FB * P], dtw, tag="c_wf")
            nc.sync.dma_start(
                c_wf[:].rearrange("p (t m) -> p t m", m=P),
                wf.rearrange("(t p) m -> p t m", p=P))
            c32 = cp.tile([P, P32_N], f32, tag="c32")
            nc.sync.dma_start(c32[:], cpack32)
            cr = cp.tile([1, PR_N], f32, tag="cr")
            nc.sync.dma_start(cr[:], rpack32)

            c_stc = c16[:, P16_STC:P16_STC + KT_CORE]
            c_sts = c16[:, P16_STS:P16_STS + KT_SHELL]
            c_stf = c16[:, P16_STF:P16_STF + KT_FB]
            c_hcl = c16[:, P16_HCL:P16_HCL + KT_HH]
            c_idn16 = c16[:, P16_IDN:P16_IDN + P]
            c_sto = c32[:, P32_STO:P32_STO + OWN_COLS]
            c_omg = c32[:, P32_OMG:P32_OMG + OWN_COLS]
            c_gmm = c32[:, P32_GMM:P32_GMM + OWN_COLS]
            c_dob = c32[:, P32_DOB:P32_DOB + 1]
            c_idn32 = c32[:, P32_IDN:P32_IDN + P]
            c_win = cr[:, PR_WIN:PR_WIN + M_CORE]
            c_con = cr[:, PR_CON:PR_CON + P]
            c_bio = cr[:, PR_BIO:PR_BIO + 512]
            c_bho = cr[:, PR_BHO:PR_BHO + 512]
            c_wou = cr[:, PR_WOU:PR_WOU + P]
            c_xb = cr[:, PR_XB:PR_XB + 1]

            # ---- PSUM row tiles (one bank each: exactly one start/stop per bank)
            ps_g = pp.tile([1, 512], f32, tag="ps_g")
            ps_s = pp.tile([1, SHELL_OWN], f32, tag="row", bufs=5)
            ps_f = pp.tile([1, P], f32, tag="row", bufs=5)
            ps_ca = pp.tile([1, 512], f32, tag="row", bufs=5)
            ps_cb = pp.tile([1, P], f32, tag="row", bufs=5)

            # ---- stage 1: shell/fb first so every core reaches allgather-1 early
            for kt in range(KT_SHELL):
                nc.tensor.matmul(ps_s[0:1, :], c_sts[:, kt:kt + 1],
                                 c_ws[:, kt * SHELL_OWN:(kt + 1) * SHELL_OWN],
                                 start=(kt == 0), stop=(kt == KT_SHELL - 1))
            for kt in range(KT_FB):
                nc.tensor.matmul(ps_f[0:1, :], c_stf[:, kt:kt + 1],
                                 c_wf[:, kt * P:(kt + 1) * P],
                                 start=(kt == 0), stop=(kt == KT_FB - 1))

            # allgather-1 shard row: slots 0-2 shell_pred chunks, slot 3 fb_pred
            # (bf16 for the wire + an f32 twin for the local pred transpose)
            shard = wk.tile([1, 512], dtw, tag="shard")
            nc.vector.memset(shard[:], 0.0)
            srow32 = wk.tile([1, 512], f32, tag="srow32")
            nc.vector.memset(srow32[:], 0.0)
            for dst in (shard, srow32):
                nc.scalar.activation(dst[0:1, 0:P], ps_s[0:1, 0:P], AF.Tanh)
                nc.scalar.activation(dst[0:1, P:2 * P], ps_s[0:1, P:2 * P], AF.Tanh)
                nc.scalar.activation(dst[0:1, 2 * P:SHELL_OWN],
                                     ps_s[0:1, 2 * P:SHELL_OWN], AF.Tanh)
                nc.scalar.activation(dst[0:1, 3 * P:4 * P], ps_f[0:1, :], AF.Tanh)

            ag1_src = dp.tile([1, 512], dtw, tag="ag1_src")
            ag1_dst = dp.tile([NCORES, 512], dtw, tag="ag1_dst")
            nc.scalar.dma_start(ag1_src[:], shard[:])
            nc.gpsimd.collective_compute(
                "AllGather", ALU.bypass, replica_groups=rg,
                ins=[ag1_src.opt()], outs=[ag1_dst.opt()])
            ag1_g = wk.tile([NCORES, 512], dtw, tag="ag1_g")
            nc.scalar.dma_start(ag1_g[:], ag1_dst[:])

            # ---- W_hh @ h partial gates (opens the ps_g accumulation)
            c_whh = wxp.tile([P, KT_HH * 512], dtw, tag="wx")
            nc.sync.dma_start(
                c_whh[:].rearrange("p (t m) -> p t m", m=512),
                whh.rearrange("(t p) m -> p t m", p=P))
            for kt in range(KT_HH):
                nc.tensor.matmul(ps_g[0:1, :], c_hcl[:, kt:kt + 1],
                                 c_whh[:, kt * 512:(kt + 1) * 512],
                                 start=(kt == 0), stop=False)

            # ---- cheap precomputes
            a_t = wk.tile([P, OWN_COLS], f32, tag="a_t")      # 1 + 0.1*omega
            nc.vector.tensor_scalar(a_t[:], c_omg, 0.1, 1.0, ALU.mult, ALU.add)
            b_t = wk.tile([P, OWN_COLS], f32, tag="b_t")      # 0.1*gamma
            nc.vector.tensor_scalar_mul(b_t[:], c_gmm, 0.1)
            st9 = wk.tile([P, OWN_COLS], f32, tag="st9")      # 0.9*state_own
            nc.vector.tensor_scalar_mul(st9[:], c_sto, 1.0 - LEAK)
            bsum = wk.tile([1, 512], f32, tag="bsum")
            nc.vector.tensor_add(bsum[:], c_bio, c_bho)
            rin = wk.tile([1, M_CORE], f32, tag="rin")        # W_in[:CORE]*x
            nc.vector.tensor_scalar(rin[:], c_win, c_xb[0:1, 0:1], None, ALU.mult)

            # ---- stage-1 core GEMV (streams while allgather-1 is in flight)
            for (kt0, nkt) in _chunks(KT_CORE, CH_CORE):
                t = wcp.tile([P, CH_CORE * M_CORE], dtw, tag="wc")
                nc.sync.dma_start(
                    t[:, : nkt * M_CORE].rearrange("p (t m) -> p t m", m=M_CORE),
                    wc[kt0 * P:(kt0 + nkt) * P, :].rearrange("(t p) m -> p t m", p=P))
                for k in range(nkt):
                    kt = kt0 + k
                    xcol = c_stc[:, kt:kt + 1]
                    nc.tensor.matmul(ps_ca[0:1, :], xcol,
                                     t[:, k * M_CORE: k * M_CORE + 512],
                                     start=(kt == 0), stop=(kt == KT_CORE - 1))
                    nc.tensor.matmul(ps_cb[0:1, :], xcol,
                                     t[:, k * M_CORE + 512:(k + 1) * M_CORE],
                                     start=(kt == 0), stop=(kt == KT_CORE - 1))

            # core_pred1 = tanh(psum + rin), kept in row layout
            crow = wk.tile([1, M_CORE], f32, tag="crow")
            nc.vector.tensor_add(crow[0:1, 0:512], ps_ca[0:1, :], rin[0:1, 0:512])
            nc.vector.tensor_add(crow[0:1, 512:M_CORE], ps_cb[0:1, :],
                                 rin[0:1, 512:M_CORE])
            nc.scalar.activation(crow[:], crow[:], AF.Tanh)

            # ---- transpose gathered shards -> [128, 32] columns (col = cs*8+j)
            ps_t3 = pp.tile([P, 4 * NCORES], dtw, tag="tr", bufs=2)
            for cs in range(4):
                nc.tensor.matmul(ps_t3[:, cs * 8:(cs + 1) * 8],
                                 ag1_g[:, cs * P:(cs + 1) * P],
                                 c_idn16[0:NCORES, 0:NCORES],
                                 is_transpose=True, start=True, stop=True)
            ag1_sb = wk.tile([P, 4 * NCORES], dtw, tag="ag1_sb")
            nc.vector.tensor_copy(ag1_sb[:], ps_t3[:])

            # ---- stage 2 weight prefetch, then the full W_ih stream behind it
            # (all issued on the sync queue with no interleaved waits, so the
            # whole remaining weight volume streams without stalling)
            wx_tiles = []
            for (kt0, nkt) in _chunks(KT_X, CH_X):
                t = wxp.tile([P, CH_X * M_CORE], dtw, tag="wx")
                nc.sync.dma_start(
                    t[:, : nkt * M_CORE].rearrange("p (t m) -> p t m", m=M_CORE),
                    wx[kt0 * P:(kt0 + nkt) * P, :].rearrange("(t p) m -> p t m", p=P))
                wx_tiles.append((kt0, nkt, t))
            c_wih = wihp.tile([P, KT_IH * 512], dtw, tag="wih")
            nc.sync.dma_start(
                c_wih[:].rearrange("p (t m) -> p t m", m=512),
                wih.rearrange("(t p) m -> p t m", p=P))

            # ---- stage 2: cross contributions (contract full shell/fb preds)
            ps_sca = pp.tile([1, 512], f32, tag="row", bufs=5)
            ps_scb = pp.tile([1, P], f32, tag="row", bufs=5)
            ps_fca = pp.tile([1, 512], f32, tag="row", bufs=5)
            ps_fcb = pp.tile([1, P], f32, tag="row", bufs=5)
            for (kt0, nkt, t) in wx_tiles:
                for k in range(nkt):
                    kt = kt0 + k
                    xcol = ag1_sb[:, kt:kt + 1]      # col = cslot*8 + j
                    if kt < 24:
                        pa, pb, st, sp = ps_sca, ps_scb, kt == 0, kt == 23
                    else:
                        pa, pb, st, sp = ps_fca, ps_fcb, kt == 24, kt == KT_X - 1
                    nc.tensor.matmul(pa[0:1, :], xcol,
                                     t[:, k * M_CORE: k * M_CORE + 512],
                                     start=st, stop=sp)
                    nc.tensor.matmul(pb[0:1, :], xcol,
                                     t[:, k * M_CORE + 512:(k + 1) * M_CORE],
                                     start=st, stop=sp)

            # core_pred += tanh(sc) + tanh(fc)   (rows)
            tsc = wk.tile([1, M_CORE], f32, tag="tsc")
            nc.scalar.activation(tsc[0:1, 0:512], ps_sca[0:1, :], AF.Tanh)
            nc.scalar.activation(tsc[0:1, 512:M_CORE], ps_scb[0:1, :], AF.Tanh)
            tfc = wk.tile([1, M_CORE], f32, tag="tfc")
            nc.scalar.activation(tfc[0:1, 0:512], ps_fca[0:1, :], AF.Tanh)
            nc.scalar.activation(tfc[0:1, 512:M_CORE], ps_fcb[0:1, :], AF.Tanh)
            nc.vector.tensor_add(crow[:], crow[:], tsc[:])
            nc.vector.tensor_add(crow[:], crow[:], tfc[:])

            # ---- transpose pred into column layout
            ps_pred = pp.tile([P, CORE_COLS], f32, tag="tr", bufs=2)
            for cslot in range(CORE_COLS):
                nc.tensor.matmul(ps_pred[:, cslot:cslot + 1],
                                 crow[0:1, cslot * P:(cslot + 1) * P],
                                 c_idn32[0:1, 0:1],
                                 is_transpose=True, start=True, stop=True)
            ps_pred2 = pp.tile([P, 4], f32, tag="tr", bufs=2)
            for cs in range(4):
                nc.tensor.matmul(ps_pred2[:, cs:cs + 1],
                                 srow32[0:1, cs * P:(cs + 1) * P],
                                 c_idn32[0:1, 0:1],
                                 is_transpose=True, start=True, stop=True)

            # ---- stage 3: leak + harmonic oscillator + dopamine (columns)
            s_own = wk.tile([P, OWN_COLS], f32, tag="s_own")
            nc.vector.scalar_tensor_tensor(s_own[:, 0:CORE_COLS],
                                           ps_pred[:], LEAK,
                                           st9[:, 0:CORE_COLS],
                                           ALU.mult, ALU.add)
            nc.vector.scalar_tensor_tensor(s_own[:, CORE_COLS:OWN_COLS],
                                           ps_pred2[:], LEAK,
                                           st9[:, CORE_COLS:OWN_COLS],
                                           ALU.mult, ALU.add)
            t2 = wk.tile([P, OWN_COLS], f32, tag="t2")
            nc.scalar.activation(t2[:], s_own[:], AF.Tanh)
            t3 = wk.tile([P, OWN_COLS], f32, tag="t3")
            nc.vector.tensor_mul(t3[:], s_own[:], a_t[:])
            nc.vector.tensor_mul(t2[:], t2[:], b_t[:])
            nc.vector.tensor_sub(s_own[:], t3[:], t2[:])
            nc.vector.tensor_scalar(s_own[:], s_own[:], c_dob[:, 0:1], None, ALU.mult)

            # ---- s to rows; emit output + allgather-2 shard
            ps_ts = pp.tile([OWN_COLS, P], f32, tag="tr", bufs=2)
            nc.tensor.matmul(ps_ts[:], s_own[:], c_idn32[:, 0:P],
                             is_transpose=True, start=True, stop=True)
            s_rows = wk.tile([OWN_COLS, P], f32, tag="s_rows")
            nc.vector.tensor_copy(s_rows[:], ps_ts[:])
            nc.scalar.dma_start(s_out, s_rows[:])
            s_rows16 = wk.tile([OWN_COLS, P], dtw, tag="s_rows16")
            nc.vector.tensor_copy(s_rows16[:], ps_ts[:])

            ag2_src = dp.tile([OWN_COLS, P], dtw, tag="ag2_src")
            ag2_dst = dp.tile([OWN_COLS * NCORES, P], dtw, tag="ag2_dst")
            nc.scalar.dma_start(ag2_src[:], s_rows16[:])
            nc.gpsimd.collective_compute(
                "AllGather", ALU.bypass, replica_groups=rg,
                ins=[ag2_src.opt()], outs=[ag2_dst.opt()])
            ag2_g = wk.tile([OWN_COLS * NCORES, P], dtw, tag="ag2_g")
            nc.scalar.dma_start(ag2_g[:], ag2_dst[:])

            ps_t4 = pp.tile([P, OWN_COLS * NCORES], dtw, tag="tr", bufs=2)
            nc.tensor.matmul(ps_t4[:], ag2_g[:],
                             c_idn16[0:OWN_COLS * NCORES, 0:OWN_COLS * NCORES],
                             is_transpose=True, start=True, stop=True)
            ag2_sb = wk.tile([P, OWN_COLS * NCORES], dtw, tag="ag2_sb")
            nc.vector.tensor_copy(ag2_sb[:], ps_t4[:])

            # ---- stage 4: gates += W_ih @ s  (weights prefetched above)
            for kt in range(KT_IH):
                nc.tensor.matmul(ps_g[0:1, :], ag2_sb[:, kt:kt + 1],
                                 c_wih[:, kt * 512:(kt + 1) * 512],
                                 start=False, stop=(kt == KT_IH - 1))

            # ---- LSTM cell for this core's 128 hidden units (row layout)
            g_row = wk.tile([1, 512], f32, tag="g_row")
            nc.vector.tensor_add(g_row[:], ps_g[0:1, :], bsum[:])
            acts = wk.tile([1, 512], f32, tag="acts")
            nc.scalar.activation(acts[0:1, 0:P], g_row[0:1, 0:P], AF.Sigmoid)
            nc.scalar.activation(acts[0:1, P:2 * P], g_row[0:1, P:2 * P], AF.Sigmoid)
            nc.scalar.activation(acts[0:1, 2 * P:3 * P], g_row[0:1, 2 * P:3 * P], AF.Tanh)
            nc.scalar.activation(acts[0:1, 3 * P:4 * P], g_row[0:1, 3 * P:4 * P], AF.Sigmoid)
            t5 = wk.tile([1, P], f32, tag="t5")
            nc.vector.tensor_mul(t5[:], acts[0:1, P:2 * P], c_con)       # f*c
            t6 = wk.tile([1, P], f32, tag="t6")
            nc.vector.tensor_mul(t6[:], acts[0:1, 0:P], acts[0:1, 2 * P:3 * P])
            c_t = wk.tile([1, P], f32, tag="c_t")
            nc.vector.tensor_add(c_t[:], t5[:], t6[:])                      # c_new
            tcn = wk.tile([1, P], f32, tag="tcn")
            nc.scalar.activation(tcn[:], c_t[:], AF.Tanh)
            h_t = wk.tile([1, P], f32, tag="h_t")
            nc.vector.tensor_mul(h_t[:], acts[0:1, 3 * P:4 * P], tcn[:])    # h_new
            hw = wk.tile([1, P], f32, tag="hw")
            nc.vector.tensor_mul(hw[:], h_t[:], c_wou)
            po_sb = wk.tile([1, 1], f32, tag="po_sb")
            nc.vector.reduce_sum(po_sb[:], hw[:], axis=AX.X)
            nc.scalar.dma_start(h_out, h_t[:])
            nc.scalar.dma_start(c_out, c_t[:])
            nc.scalar.dma_start(po_out, po_sb[:])

    nc.compile()
    return nc


_NC_CACHE = None


def _get_nc():
    global _NC_CACHE
    if _NC_CACHE is None:
        _NC_CACHE = _build_nc()
    return _NC_CACHE


def _unshard(results, b_out):
    s_full = np.zeros(R, np.float32)
    for i in range(NCORES):
        flat = results[i]["s_out"].reshape(-1)
        s_full[C_OFF[i]:C_OFF[i] + CC[i]] = flat[:CC[i]]
        s_full[CORE + S_OFF[i]:CORE + S_OFF[i] + SC[i]] = flat[640:640 + SC[i]]
        s_full[CORE + SHELL + F_OFF[i]:CORE + SHELL + F_OFF[i] + FC[i]] = \
            flat[1024:1024 + FC[i]]
    h_new = np.concatenate([results[i]["h_out"][0] for i in range(NCORES)])
    c_new = np.concatenate([results[i]["c_out"][0] for i in range(NCORES)])
    out = np.float32(sum(float(results[i]["po_out"][0, 0]) for i in range(NCORES)))
    output = (out + b_out.astype(np.float32)).reshape(1, 1)
    return (output.astype(np.float32),
            s_full.reshape(1, R),
            h_new.reshape(1, 1, H).astype(np.float32),
            c_new.reshape(1, 1, H).astype(np.float32))


def run(inputs, trace=False):
    from concourse import bass_utils
    nc = _get_nc()
    in_maps = _prep_in_maps(inputs)
    kw = {}
    if trace:
        kw = dict(trace=True, trace_cores=list(range(NCORES)))
    res = bass_utils.run_bass_kernel_spmd(
        nc, in_maps, core_ids=list(range(NCORES)), **kw)
    outs = _unshard(res.results, np.asarray(inputs["b_out"], np.float32))
    return outs, res


def kernel(**inputs):
    outs, _ = run(inputs, trace=False)
    return outs
